# revision 32
# baseline (speedup 1.0000x reference)
"""CentroidLayer (retrieval kNN) Bass/Tile kernel for 8 trn2 NeuronCores.

Sharding: data-parallel over batch B (4096 -> 512 rows/core); centroids
replicated (they are module weights, so their layout prep — transpose to
contraction-major and the per-centroid squared-norm rows — happens on the
host once, like any weight pre-packing).

Per core:
  - W = -2*c^T  (host-prepped, f32) is cast-DMA'd to bf16 SBUF, fully
    resident (8 MB); x is cast-loaded and PE-transposed to D-major
  - d2 = x2 + c2 - 2*x@c^T accumulates in PSUM over 8 K=128 bf16 matmuls
    plus one K=4 correction matmul with rows [1,1,x2_hi,x2_lo] x
    [c2_hi,c2_lo,1,1] (hi/lo bf16 splits keep f32-level accuracy); x2 is
    computed on device via ACT Square+accumulate
  - the matmul loop shares each stationary operand across both j-groups of
    a resident pair (fewer LDWEIGHTS, denser PE stream)
  - grouped min over the 4 centroids per class on DVE (tensor_reduce min)
  - y = -sqrt(d2min) on ACT + DVE negate
  - soft_accept = sigmoid(min_dist*(-1/temp) + max_ac/temp) with a
    Newton-refined sqrt for the [128,1] min-distance column (ACT sqrt
    tables are low precision; the sigmoid is sensitive to absolute error)
Outputs [512, 1001] f32 per core are concatenated on host.
"""

import math
from contextlib import ExitStack

import numpy as np
import ml_dtypes

import concourse.bacc as bacc
import concourse.bass as bass
import concourse.mybir as mybir
import concourse.tile as tile
from concourse.bass_utils import run_bass_kernel_spmd
from concourse.masks import make_identity

F32 = mybir.dt.float32
BF16 = mybir.dt.bfloat16
AF = mybir.ActivationFunctionType
ALU = mybir.AluOpType
AX = mybir.AxisListType

N_CORES = 8
B, D = 4096, 1024
C_CLASSES, NPC = 1000, 4
CN = C_CLASSES * NPC
AC_STD_LIM = 5.0
GARBAGE_C2 = 1.0e9


def build_nc(b_loc=B // N_CORES, cn=CN, d=D, npc=NPC, n_classes=C_CLASSES,
             n_cores=N_CORES):
    """Build + compile the per-core Bass module (SPMD: same program on all
    cores; only the x shard differs)."""
    assert b_loc % 128 == 0 and d % 128 == 0 and cn % npc == 0
    nb, nd = b_loc // 128, d // 128
    JG = math.ceil(cn / 512)          # j-groups of 512 centroid rows
    assert JG % 2 == 0
    cnp = JG * 512                    # padded centroid rows
    n_out = n_classes + 1

    nc = bacc.Bacc("TRN2", target_bir_lowering=False, debug=False,
                   enable_asserts=False, num_devices=n_cores)

    x_d = nc.dram_tensor("x", [b_loc, d], F32, kind="ExternalInput").ap()
    w_d = nc.dram_tensor("wt", [d, cnp], BF16, kind="ExternalInput").ap()
    c2_d = nc.dram_tensor("c2r", [4, cnp], BF16, kind="ExternalInput").ap()
    a_d = nc.dram_tensor("acol", [128, 1], F32, kind="ExternalInput").ap()
    bi_d = nc.dram_tensor("bcol", [128, 1], F32, kind="ExternalInput").ap()
    out_d = nc.dram_tensor("out", [b_loc, n_out], F32, kind="ExternalOutput").ap()

    with tile.TileContext(nc) as tc, ExitStack() as ctx:
        const = ctx.enter_context(tc.tile_pool(name="const", bufs=1))
        sq_pool = ctx.enter_context(tc.tile_pool(name="sq", bufs=2))
        small = ctx.enter_context(tc.tile_pool(name="small", bufs=2))
        out_pool = ctx.enter_context(tc.tile_pool(name="otile", bufs=1))
        trp_cm = tc.tile_pool(name="trp", bufs=2, space="PSUM")
        smp_cm = tc.tile_pool(name="smp", bufs=1, space="PSUM")
        trp = trp_cm.__enter__()
        smp = smp_cm.__enter__()

        ident = const.tile([128, 128], BF16)
        make_identity(nc, ident)
        acol = const.tile([128, 1], F32)
        nc.sync.dma_start(acol[:], a_d)
        bcol = const.tile([128, 1], F32)
        nc.sync.dma_start(bcol[:], bi_d)
        corr_rhs = const.tile([4, cnp], BF16)
        nc.sync.dma_start(corr_rhs[:], c2_d)

        # ---- x prep: per-b-tile load (HWDGE) + DVE cast, x2, transpose ----
        xf = const.tile([128, nb * d], F32)        # bt-major natural x, f32
        xn = const.tile([128, nb * d], BF16)       # same, cast to bf16
        x2c = const.tile([128, nb], F32)           # x2 per b-tile column
        corr_lhsT = const.tile([4, nb * 128], BF16)
        xTt = []                                   # per-b-tile [d-part, q*128]
        for t in range(nb):
            nc.sync.dma_start(out=xf[:, t * d:(t + 1) * d],
                              in_=x_d[t * 128:(t + 1) * 128, :])
            nc.vector.tensor_copy(xn[:, t * d:(t + 1) * d],
                                  xf[:, t * d:(t + 1) * d])
            xsq = sq_pool.tile([128, d], F32, tag="xsq")
            nc.scalar.activation(xsq[:], xf[:, t * d:(t + 1) * d], AF.Square,
                                 accum_out=x2c[:, t:t + 1])
            # cols [1, 1, x2_hi, x2_lo], then one 128x4 -> 4x128 transpose
            hl = small.tile([128, 4], BF16, tag="hl")
            nc.vector.memset(hl[:, 0:2], 1.0)
            nc.vector.tensor_copy(hl[:, 2:3], x2c[:, t:t + 1])
            hf = small.tile([128, 1], F32, tag="hf")
            nc.vector.tensor_copy(hf[:], hl[:, 2:3])
            nc.vector.tensor_tensor(out=hf[:], in0=x2c[:, t:t + 1],
                                    in1=hf[:], op=ALU.subtract)
            nc.vector.tensor_copy(hl[:, 3:4], hf[:])
            sp2 = smp.tile([8, 128], BF16, tag="sp")
            nc.tensor.transpose(sp2[0:4, :], hl[:], ident[:])
            nc.vector.tensor_copy(corr_lhsT[:, t * 128:(t + 1) * 128],
                                  sp2[0:4, :])
            # transpose this b-tile to D-major: 8 blocks into one psum bank
            tp = trp.tile([128, nd * 128], BF16, tag="trp")
            for q in range(nd):
                nc.tensor.transpose(
                    tp[:, q * 128:(q + 1) * 128],
                    xn[:, t * d + q * 128: t * d + (q + 1) * 128], ident[:])
            xt = const.tile([128, nd * 128], BF16, tag=f"xTt{t}",
                            name=f"xTt{t}")
            nc.vector.tensor_copy(xt[:], tp[:])
            xTt.append(xt)

        smp_cm.__exit__(None, None, None)
        trp_cm.__exit__(None, None, None)
        mmp = ctx.enter_context(tc.tile_pool(name="mmp", bufs=6, space="PSUM"))

        mins = [const.tile([128, JG * 128], F32, tag=f"mins{t}",
                           name=f"mins{t}") for t in range(nb)]
        otiles = [out_pool.tile([128, n_out + 7], F32, tag=f"o{t}",
                                name=f"o{t}") for t in range(nb)]

        # ---- main loop: quads of j-groups; W tiles stay resident.  Each
        # stationary load serves 4 matmuls; the two W pair-loads of a quad
        # ride different HWDGE rings so they arrive concurrently ----
        n_pairs = JG // 2
        ctps = []
        for pr in range(n_pairs):
            j0 = pr * 1024
            ctp = const.tile([128, nd * 1024], BF16, tag=f"ct{pr}",
                             name=f"ct{pr}")
            eng = nc.scalar if pr % 2 == 0 else nc.sync
            eng.dma_start(
                out=ctp[:].rearrange("p (q j) -> p q j", j=1024),
                in_=w_d.rearrange("(q p) j -> p q j", p=128)[:, :, j0:j0 + 1024],
            )
            ctps.append(ctp)
        n_sg = (n_pairs + 1) // 2
        for qd in range(n_sg):
            sg_pairs = min(2, n_pairs - 2 * qd)
            nh = 2 * sg_pairs
            for t in range(nb):
                pms = [mmp.tile([128, 512], F32, tag="mm",
                                name=f"pm{qd}_{t}_{h}") for h in range(nh)]
                for q in range(nd):
                    for h in range(nh):
                        ctp = ctps[2 * qd + h // 2]
                        hh = h % 2
                        nc.tensor.matmul(
                            pms[h][:],
                            lhsT=xTt[t][:, q * 128:(q + 1) * 128],
                            rhs=ctp[:, q * 1024 + hh * 512: q * 1024 + (hh + 1) * 512],
                            start=(q == 0), stop=False)
                for h in range(nh):
                    jg = 4 * qd + h
                    nc.tensor.matmul(
                        pms[h][:], lhsT=corr_lhsT[:, t * 128:(t + 1) * 128],
                        rhs=corr_rhs[:, jg * 512:(jg + 1) * 512],
                        start=False, stop=True)
                    nc.vector.tensor_reduce(
                        out=mins[t][:, jg * 128:(jg + 1) * 128],
                        in_=pms[h][:].rearrange("p (c n) -> p c n", n=npc),
                        axis=AX.X, op=ALU.min)
                # stream y = -sqrt(d2min) for this quad's classes now, so the
                # kernel tail only handles the last block + soft column
                c_lo = qd * 512
                c_hi = min(qd * 512 + sg_pairs * 256, n_classes)
                if c_hi > c_lo:
                    nc.scalar.activation(otiles[t][:, c_lo:c_hi],
                                         mins[t][:, c_lo:c_hi], AF.Sqrt,
                                         bias=0.0, scale=1.0)
                    nc.vector.tensor_scalar_mul(otiles[t][:, c_lo:c_hi],
                                                otiles[t][:, c_lo:c_hi], -1.0)
                if qd == n_sg - 1:
                    eng = nc.sync if t % 2 == 0 else nc.scalar
                    eng.dma_start(out_d[t * 128:(t + 1) * 128, 0:n_classes],
                                  otiles[t][:, 0:n_out - 1])

        # ---- epilogue: just the soft_accept column ----
        ssall = const.tile([128, nb], F32)
        softall = const.tile([128, nb], F32)
        for t in range(nb):
            # min over classes (garbage classes hold ~1e9, never win)
            mmc = small.tile([128, 1], F32, tag="mmc")
            nc.vector.tensor_reduce(out=mmc[:], in_=mins[t][:], axis=AX.X,
                                    op=ALU.min)
            s0 = small.tile([128, 1], F32, tag="s0")
            nc.scalar.activation(s0[:], mmc[:], AF.Sqrt, bias=0.0, scale=1.0)
            # one Newton step: s1 = (s0 + v/s0)/2  (ACT sqrt is low precision)
            rc = small.tile([128, 1], F32, tag="rc")
            nc.vector.reciprocal(rc[:], s0[:])
            t1 = small.tile([128, 1], F32, tag="t1")
            nc.vector.tensor_tensor(out=t1[:], in0=mmc[:], in1=rc[:],
                                    op=ALU.mult)
            nc.vector.tensor_tensor(out=t1[:], in0=s0[:], in1=t1[:],
                                    op=ALU.add)
            nc.vector.tensor_scalar_mul(ssall[:, t:t + 1], t1[:], 0.5)
        # soft_accept = sigmoid(min_dist * (-1/temp) + max_ac/temp)
        nc.scalar.activation(softall[:], ssall[:], AF.Sigmoid,
                             bias=acol[:], scale=bcol[:])
        for t in range(nb):
            nc.gpsimd.dma_start(
                out_d[t * 128:(t + 1) * 128, n_classes:n_classes + 1],
                softall[:, t:t + 1])

    nc.compile()
    return nc


_CACHE = {}


def _get_nc():
    if "nc" not in _CACHE:
        _CACHE["nc"] = build_nc()
    return _CACHE["nc"]


def _prep_centroids(c):
    """Weight pre-packing: W = -2*c^T (zero-padded to 4096 cols) and the
    [c2_hi, c2_lo, 1, 1] bf16 correction rows (1e9 on padded classes)."""
    cnp = math.ceil(CN / 512) * 512
    w = np.zeros((D, cnp), dtype=ml_dtypes.bfloat16)
    w[:, :CN] = (np.ascontiguousarray(c.T) * np.float32(-2.0)).astype(
        ml_dtypes.bfloat16)
    c2 = (c.astype(np.float64) ** 2).sum(1).astype(np.float32)
    c2f = np.full(cnp, GARBAGE_C2, dtype=np.float32)
    c2f[:CN] = c2
    c2h = c2f.astype(ml_dtypes.bfloat16)
    c2l = (c2f - c2h.astype(np.float32)).astype(ml_dtypes.bfloat16)
    ones = np.ones(cnp, dtype=ml_dtypes.bfloat16)
    c2r = np.stack([c2h, c2l, ones, ones], axis=0)
    return w, c2r


def _host_prep(x, centroids, std_scale, ac_temp, running_mean, running_var):
    x = np.asarray(x, dtype=np.float32)
    c = np.asarray(centroids, dtype=np.float32).reshape(CN, D)
    std_scale = np.float32(np.asarray(std_scale))
    ac_temp = np.float32(np.asarray(ac_temp))
    running_mean = np.float32(np.asarray(running_mean))
    running_var = np.float32(np.asarray(running_var))

    clip = np.float32(min(max(float(std_scale), 0.0), AC_STD_LIM))
    max_ac = np.float32(running_mean + clip * np.float32(np.sqrt(running_var)))
    acol = np.full((128, 1), np.float32(max_ac / ac_temp), dtype=np.float32)
    bcol = np.full((128, 1), np.float32(-1.0 / ac_temp), dtype=np.float32)

    w, c2r = _prep_centroids(c)

    b_loc = B // N_CORES
    in_maps = []
    for i in range(N_CORES):
        in_maps.append({
            "x": np.ascontiguousarray(x[i * b_loc:(i + 1) * b_loc]),
            "wt": w,
            "c2r": c2r,
            "acol": acol,
            "bcol": bcol,
        })
    return in_maps


def run_spmd(in_maps, trace=False, **kw):
    nc = _get_nc()
    return run_bass_kernel_spmd(nc, in_maps, list(range(N_CORES)),
                                trace=trace, **kw)


def kernel(x, centroids, std_scale, ac_temp, running_mean, running_var):
    in_maps = _host_prep(x, centroids, std_scale, ac_temp,
                         running_mean, running_var)
    res = run_spmd(in_maps)
    return np.concatenate([res.results[i]["out"] for i in range(N_CORES)],
                          axis=0)


# revision 33
# speedup vs baseline: 1.0723x; 1.0723x over previous
"""CentroidLayer (retrieval kNN) Bass/Tile kernel for 8 trn2 NeuronCores.

Sharding: data-parallel over batch B (4096 -> 512 rows/core); centroids
replicated (they are module weights, so their layout prep — transpose to
contraction-major and the per-centroid squared-norm rows — happens on the
host once, like any weight pre-packing).

Per core:
  - W = -2*c^T  (host-prepped, f32) is cast-DMA'd to bf16 SBUF, fully
    resident (8 MB); x is cast-loaded and PE-transposed to D-major
  - d2 = x2 + c2 - 2*x@c^T accumulates in PSUM over 8 K=128 bf16 matmuls
    plus one K=4 correction matmul with rows [1,1,x2_hi,x2_lo] x
    [c2_hi,c2_lo,1,1] (hi/lo bf16 splits keep f32-level accuracy); x2 is
    computed on device via ACT Square+accumulate
  - the matmul loop shares each stationary operand across both j-groups of
    a resident pair (fewer LDWEIGHTS, denser PE stream)
  - grouped min over the 4 centroids per class on DVE (tensor_reduce min)
  - y = -sqrt(d2min) on ACT + DVE negate
  - soft_accept = sigmoid(min_dist*(-1/temp) + max_ac/temp) with a
    Newton-refined sqrt for the [128,1] min-distance column (ACT sqrt
    tables are low precision; the sigmoid is sensitive to absolute error)
Outputs [512, 1001] f32 per core are concatenated on host.
"""

import math
from contextlib import ExitStack

import numpy as np
import ml_dtypes

import concourse.bacc as bacc
import concourse.bass as bass
import concourse.mybir as mybir
import concourse.tile as tile
from concourse.bass_utils import run_bass_kernel_spmd
from concourse.masks import make_identity

F32 = mybir.dt.float32
BF16 = mybir.dt.bfloat16
AF = mybir.ActivationFunctionType
ALU = mybir.AluOpType
AX = mybir.AxisListType

N_CORES = 8
B, D = 4096, 1024
C_CLASSES, NPC = 1000, 4
CN = C_CLASSES * NPC
AC_STD_LIM = 5.0
GARBAGE_C2 = 1.0e9


def build_nc(b_loc=B // N_CORES, cn=CN, d=D, npc=NPC, n_classes=C_CLASSES,
             n_cores=N_CORES):
    """Build + compile the per-core Bass module (SPMD: same program on all
    cores; only the x shard differs)."""
    assert b_loc % 128 == 0 and d % 128 == 0 and cn % npc == 0
    nb, nd = b_loc // 128, d // 128
    JG = math.ceil(cn / 512)          # j-groups of 512 centroid rows
    assert JG % 2 == 0
    cnp = JG * 512                    # padded centroid rows
    n_out = n_classes + 1

    nc = bacc.Bacc("TRN2", target_bir_lowering=False, debug=False,
                   enable_asserts=False, num_devices=n_cores)

    x_d = nc.dram_tensor("x", [b_loc, d], F32, kind="ExternalInput").ap()
    w_d = nc.dram_tensor("wt", [d, cnp], BF16, kind="ExternalInput").ap()
    c2_d = nc.dram_tensor("c2r", [4, cnp], BF16, kind="ExternalInput").ap()
    a_d = nc.dram_tensor("acol", [128, 1], F32, kind="ExternalInput").ap()
    bi_d = nc.dram_tensor("bcol", [128, 1], F32, kind="ExternalInput").ap()
    out_d = nc.dram_tensor("out", [b_loc, n_out], F32, kind="ExternalOutput").ap()

    with tile.TileContext(nc) as tc, ExitStack() as ctx:
        const = ctx.enter_context(tc.tile_pool(name="const", bufs=1))
        sq_pool = ctx.enter_context(tc.tile_pool(name="sq", bufs=2))
        small = ctx.enter_context(tc.tile_pool(name="small", bufs=2))
        out_pool = ctx.enter_context(tc.tile_pool(name="otile", bufs=1))
        trp = ctx.enter_context(tc.tile_pool(name="trp", bufs=2, space="PSUM"))
        mmp = ctx.enter_context(tc.tile_pool(name="mmp", bufs=5, space="PSUM"))
        smp = ctx.enter_context(tc.tile_pool(name="smp", bufs=1, space="PSUM"))

        ident = const.tile([128, 128], BF16)
        make_identity(nc, ident)
        acol = const.tile([128, 1], F32)
        nc.sync.dma_start(acol[:], a_d)
        bcol = const.tile([128, 1], F32)
        nc.sync.dma_start(bcol[:], bi_d)
        corr_rhs = const.tile([4, cnp], BF16)
        nc.sync.dma_start(corr_rhs[:], c2_d)

        # ---- x prep: per-b-tile load (HWDGE) + DVE cast, x2, transpose ----
        xf = const.tile([128, nb * d], F32)        # bt-major natural x, f32
        xn = const.tile([128, nb * d], BF16)       # same, cast to bf16
        x2c = const.tile([128, nb], F32)           # x2 per b-tile column
        corr_lhsT = const.tile([4, nb * 128], BF16)
        xTt = []                                   # per-b-tile [d-part, q*128]
        for t in range(nb):
            nc.sync.dma_start(out=xf[:, t * d:(t + 1) * d],
                              in_=x_d[t * 128:(t + 1) * 128, :])
            nc.vector.tensor_copy(xn[:, t * d:(t + 1) * d],
                                  xf[:, t * d:(t + 1) * d])
            xsq = sq_pool.tile([128, d], F32, tag="xsq")
            nc.scalar.activation(xsq[:], xf[:, t * d:(t + 1) * d], AF.Square,
                                 accum_out=x2c[:, t:t + 1])
            # cols [1, 1, x2_hi, x2_lo], then one 128x4 -> 4x128 transpose
            hl = small.tile([128, 4], BF16, tag="hl")
            nc.vector.memset(hl[:, 0:2], 1.0)
            nc.vector.tensor_copy(hl[:, 2:3], x2c[:, t:t + 1])
            hf = small.tile([128, 1], F32, tag="hf")
            nc.vector.tensor_copy(hf[:], hl[:, 2:3])
            nc.vector.tensor_tensor(out=hf[:], in0=x2c[:, t:t + 1],
                                    in1=hf[:], op=ALU.subtract)
            nc.vector.tensor_copy(hl[:, 3:4], hf[:])
            sp2 = smp.tile([8, 128], BF16, tag="sp")
            nc.tensor.transpose(sp2[0:4, :], hl[:], ident[:])
            nc.vector.tensor_copy(corr_lhsT[:, t * 128:(t + 1) * 128],
                                  sp2[0:4, :])
            # transpose this b-tile to D-major: 8 blocks into one psum bank
            tp = trp.tile([128, nd * 128], BF16, tag="trp")
            for q in range(nd):
                nc.tensor.transpose(
                    tp[:, q * 128:(q + 1) * 128],
                    xn[:, t * d + q * 128: t * d + (q + 1) * 128], ident[:])
            xt = const.tile([128, nd * 128], BF16, tag=f"xTt{t}",
                            name=f"xTt{t}")
            nc.vector.tensor_copy(xt[:], tp[:])
            xTt.append(xt)

        mins = [const.tile([128, JG * 128], F32, tag=f"mins{t}",
                           name=f"mins{t}") for t in range(nb)]
        otiles = [out_pool.tile([128, n_out + 7], F32, tag=f"o{t}",
                                name=f"o{t}") for t in range(nb)]

        # ---- main loop: pairs of j-groups; W tiles stay resident ----
        n_pairs = JG // 2
        for pr in range(n_pairs):
            j0 = pr * 1024
            ctp = const.tile([128, nd * 1024], BF16, tag=f"ct{pr}",
                             name=f"ct{pr}")
            # W rides the ACT HWDGE ring so the small const loads on the SP
            # ring never delay it
            nc.scalar.dma_start(
                out=ctp[:].rearrange("p (q j) -> p q j", j=1024),
                in_=w_d.rearrange("(q p) j -> p q j", p=128)[:, :, j0:j0 + 1024],
            )
            for t in range(nb):
                pms = [mmp.tile([128, 512], F32, tag="mm", name=f"pm{pr}_{t}_{h}")
                       for h in range(2)]
                for q in range(nd):
                    # one stationary load serves both j-groups of the pair
                    for h in range(2):
                        nc.tensor.matmul(
                            pms[h][:],
                            lhsT=xTt[t][:, q * 128:(q + 1) * 128],
                            rhs=ctp[:, q * 1024 + h * 512: q * 1024 + (h + 1) * 512],
                            start=(q == 0), stop=False)
                for h in range(2):
                    jg = 2 * pr + h
                    nc.tensor.matmul(
                        pms[h][:], lhsT=corr_lhsT[:, t * 128:(t + 1) * 128],
                        rhs=corr_rhs[:, jg * 512:(jg + 1) * 512],
                        start=False, stop=True)
                    nc.vector.tensor_reduce(
                        out=mins[t][:, jg * 128:(jg + 1) * 128],
                        in_=pms[h][:].rearrange("p (c n) -> p c n", n=npc),
                        axis=AX.X, op=ALU.min)
                # stream y = -sqrt(d2min) for this pair's classes now, so the
                # kernel tail only handles the last block + soft column
                c_lo = pr * 256
                c_hi = min((pr + 1) * 256, n_classes)
                if c_hi > c_lo:
                    nc.scalar.activation(otiles[t][:, c_lo:c_hi],
                                         mins[t][:, c_lo:c_hi], AF.Sqrt,
                                         bias=0.0, scale=1.0)
                    nc.vector.tensor_scalar_mul(otiles[t][:, c_lo:c_hi],
                                                otiles[t][:, c_lo:c_hi], -1.0)
                if pr == n_pairs - 1:
                    eng = nc.sync if t % 2 == 0 else nc.scalar
                    eng.dma_start(out_d[t * 128:(t + 1) * 128, 0:n_classes],
                                  otiles[t][:, 0:n_classes])

        # ---- epilogue: just the soft_accept column ----
        ssall = const.tile([128, nb], F32)
        softall = const.tile([128, nb], F32)
        for t in range(nb):
            # min over classes (garbage classes hold ~1e9, never win)
            mmc = small.tile([128, 1], F32, tag="mmc")
            nc.vector.tensor_reduce(out=mmc[:], in_=mins[t][:], axis=AX.X,
                                    op=ALU.min)
            s0 = small.tile([128, 1], F32, tag="s0")
            nc.scalar.activation(s0[:], mmc[:], AF.Sqrt, bias=0.0, scale=1.0)
            # one Newton step: s1 = (s0 + v/s0)/2  (ACT sqrt is low precision)
            rc = small.tile([128, 1], F32, tag="rc")
            nc.vector.reciprocal(rc[:], s0[:])
            t1 = small.tile([128, 1], F32, tag="t1")
            nc.vector.tensor_tensor(out=t1[:], in0=mmc[:], in1=rc[:],
                                    op=ALU.mult)
            nc.vector.tensor_tensor(out=t1[:], in0=s0[:], in1=t1[:],
                                    op=ALU.add)
            nc.vector.tensor_scalar_mul(ssall[:, t:t + 1], t1[:], 0.5)
        # soft_accept = sigmoid(min_dist * (-1/temp) + max_ac/temp)
        nc.scalar.activation(softall[:], ssall[:], AF.Sigmoid,
                             bias=acol[:], scale=bcol[:])
        for t in range(nb):
            nc.gpsimd.dma_start(
                out_d[t * 128:(t + 1) * 128, n_classes:n_classes + 1],
                softall[:, t:t + 1])

    nc.compile()
    return nc


_CACHE = {}


def _get_nc():
    if "nc" not in _CACHE:
        _CACHE["nc"] = build_nc()
    return _CACHE["nc"]


def _prep_centroids(c):
    """Weight pre-packing: W = -2*c^T (zero-padded to 4096 cols) and the
    [c2_hi, c2_lo, 1, 1] bf16 correction rows (1e9 on padded classes)."""
    cnp = math.ceil(CN / 512) * 512
    w = np.zeros((D, cnp), dtype=ml_dtypes.bfloat16)
    w[:, :CN] = (np.ascontiguousarray(c.T) * np.float32(-2.0)).astype(
        ml_dtypes.bfloat16)
    c2 = (c.astype(np.float64) ** 2).sum(1).astype(np.float32)
    c2f = np.full(cnp, GARBAGE_C2, dtype=np.float32)
    c2f[:CN] = c2
    c2h = c2f.astype(ml_dtypes.bfloat16)
    c2l = (c2f - c2h.astype(np.float32)).astype(ml_dtypes.bfloat16)
    ones = np.ones(cnp, dtype=ml_dtypes.bfloat16)
    c2r = np.stack([c2h, c2l, ones, ones], axis=0)
    return w, c2r


def _host_prep(x, centroids, std_scale, ac_temp, running_mean, running_var):
    x = np.asarray(x, dtype=np.float32)
    c = np.asarray(centroids, dtype=np.float32).reshape(CN, D)
    std_scale = np.float32(np.asarray(std_scale))
    ac_temp = np.float32(np.asarray(ac_temp))
    running_mean = np.float32(np.asarray(running_mean))
    running_var = np.float32(np.asarray(running_var))

    clip = np.float32(min(max(float(std_scale), 0.0), AC_STD_LIM))
    max_ac = np.float32(running_mean + clip * np.float32(np.sqrt(running_var)))
    acol = np.full((128, 1), np.float32(max_ac / ac_temp), dtype=np.float32)
    bcol = np.full((128, 1), np.float32(-1.0 / ac_temp), dtype=np.float32)

    w, c2r = _prep_centroids(c)

    b_loc = B // N_CORES
    in_maps = []
    for i in range(N_CORES):
        in_maps.append({
            "x": np.ascontiguousarray(x[i * b_loc:(i + 1) * b_loc]),
            "wt": w,
            "c2r": c2r,
            "acol": acol,
            "bcol": bcol,
        })
    return in_maps


def run_spmd(in_maps, trace=False, **kw):
    nc = _get_nc()
    return run_bass_kernel_spmd(nc, in_maps, list(range(N_CORES)),
                                trace=trace, **kw)


def kernel(x, centroids, std_scale, ac_temp, running_mean, running_var):
    in_maps = _host_prep(x, centroids, std_scale, ac_temp,
                         running_mean, running_var)
    res = run_spmd(in_maps)
    return np.concatenate([res.results[i]["out"] for i in range(N_CORES)],
                          axis=0)


# revision 34
# speedup vs baseline: 1.1181x; 1.0428x over previous
"""CentroidLayer (retrieval kNN) Bass/Tile kernel for 8 trn2 NeuronCores.

Sharding: data-parallel over batch B (4096 -> 512 rows/core); centroids
replicated (they are module weights, so their layout prep — transpose to
contraction-major and the per-centroid squared-norm rows — happens on the
host once, like any weight pre-packing).

Per core:
  - W = -2*c^T  (host-prepped, f32) is cast-DMA'd to bf16 SBUF, fully
    resident (8 MB); x is cast-loaded and PE-transposed to D-major
  - d2 = x2 + c2 - 2*x@c^T accumulates in PSUM over 8 K=128 bf16 matmuls
    plus one K=4 correction matmul with rows [1,1,x2_hi,x2_lo] x
    [c2_hi,c2_lo,1,1] (hi/lo bf16 splits keep f32-level accuracy); x2 is
    computed on device via ACT Square+accumulate
  - the matmul loop shares each stationary operand across both j-groups of
    a resident pair (fewer LDWEIGHTS, denser PE stream)
  - grouped min over the 4 centroids per class on DVE (tensor_reduce min)
  - y = -sqrt(d2min) on ACT + DVE negate
  - soft_accept = sigmoid(min_dist*(-1/temp) + max_ac/temp) with a
    Newton-refined sqrt for the [128,1] min-distance column (ACT sqrt
    tables are low precision; the sigmoid is sensitive to absolute error)
Outputs [512, 1001] f32 per core are concatenated on host.
"""

import math
from contextlib import ExitStack

import numpy as np
import ml_dtypes

import concourse.bacc as bacc
import concourse.bass as bass
import concourse.mybir as mybir
import concourse.tile as tile
from concourse.bass_utils import run_bass_kernel_spmd
from concourse.masks import make_identity

F32 = mybir.dt.float32
BF16 = mybir.dt.bfloat16
AF = mybir.ActivationFunctionType
ALU = mybir.AluOpType
AX = mybir.AxisListType

N_CORES = 8
B, D = 4096, 1024
C_CLASSES, NPC = 1000, 4
CN = C_CLASSES * NPC
AC_STD_LIM = 5.0
GARBAGE_C2 = 1.0e9


def build_nc(b_loc=B // N_CORES, cn=CN, d=D, npc=NPC, n_classes=C_CLASSES,
             n_cores=N_CORES):
    """Build + compile the per-core Bass module (SPMD: same program on all
    cores; only the x shard differs)."""
    assert b_loc % 128 == 0 and d % 128 == 0 and cn % npc == 0
    nb, nd = b_loc // 128, d // 128
    JG = math.ceil(cn / 512)          # j-groups of 512 centroid rows
    assert JG % 2 == 0
    cnp = JG * 512                    # padded centroid rows
    n_out = n_classes + 1

    nc = bacc.Bacc("TRN2", target_bir_lowering=False, debug=False,
                   enable_asserts=False, num_devices=n_cores)

    x_d = nc.dram_tensor("x", [b_loc, d], F32, kind="ExternalInput").ap()
    w_d = nc.dram_tensor("wt", [d, cnp], BF16, kind="ExternalInput").ap()
    c2_d = nc.dram_tensor("c2r", [4, cnp], BF16, kind="ExternalInput").ap()
    a_d = nc.dram_tensor("acol", [128, 1], F32, kind="ExternalInput").ap()
    bi_d = nc.dram_tensor("bcol", [128, 1], F32, kind="ExternalInput").ap()
    out_d = nc.dram_tensor("out", [b_loc, n_out], F32, kind="ExternalOutput").ap()

    with tile.TileContext(nc) as tc, ExitStack() as ctx:
        const = ctx.enter_context(tc.tile_pool(name="const", bufs=1))
        sq_pool = ctx.enter_context(tc.tile_pool(name="sq", bufs=2))
        small = ctx.enter_context(tc.tile_pool(name="small", bufs=2))
        out_pool = ctx.enter_context(tc.tile_pool(name="otile", bufs=1))
        trp = ctx.enter_context(tc.tile_pool(name="trp", bufs=2, space="PSUM"))
        mmp = ctx.enter_context(tc.tile_pool(name="mmp", bufs=5, space="PSUM"))
        smp = ctx.enter_context(tc.tile_pool(name="smp", bufs=1, space="PSUM"))

        ident = const.tile([128, 128], BF16)
        make_identity(nc, ident)
        acol = const.tile([128, 1], F32)
        nc.sync.dma_start(acol[:], a_d)
        bcol = const.tile([128, 1], F32)
        nc.sync.dma_start(bcol[:], bi_d)
        corr_rhs = const.tile([4, cnp], BF16)
        nc.sync.dma_start(corr_rhs[:], c2_d)

        # ---- x prep: per-b-tile cast-load, x2 (+hi/lo), transpose ----
        xn = const.tile([128, nb * d], BF16)       # bt-major natural x, bf16
        x2c = const.tile([128, nb], F32)           # x2 per b-tile column
        corr_lhsT = const.tile([4, nb * 128], BF16)
        xTt = []                                   # per-b-tile [d-part, q*128]
        for t in range(nb):
            nc.gpsimd.dma_start(out=xn[:, t * d:(t + 1) * d],
                                in_=x_d[t * 128:(t + 1) * 128, :])
            xsq = sq_pool.tile([128, d], F32, tag="xsq")
            nc.scalar.activation(xsq[:], xn[:, t * d:(t + 1) * d], AF.Square,
                                 accum_out=x2c[:, t:t + 1])
            # cols [1, 1, x2_hi, x2_lo], then one 128x4 -> 4x128 transpose
            hl = small.tile([128, 4], BF16, tag="hl")
            nc.vector.memset(hl[:, 0:2], 1.0)
            nc.vector.tensor_copy(hl[:, 2:3], x2c[:, t:t + 1])
            hf = small.tile([128, 1], F32, tag="hf")
            nc.vector.tensor_copy(hf[:], hl[:, 2:3])
            nc.vector.tensor_tensor(out=hf[:], in0=x2c[:, t:t + 1],
                                    in1=hf[:], op=ALU.subtract)
            nc.vector.tensor_copy(hl[:, 3:4], hf[:])
            sp2 = smp.tile([8, 128], BF16, tag="sp")
            nc.tensor.transpose(sp2[0:4, :], hl[:], ident[:])
            nc.vector.tensor_copy(corr_lhsT[:, t * 128:(t + 1) * 128],
                                  sp2[0:4, :])
            # transpose this b-tile to D-major: 8 blocks into one psum bank
            tp = trp.tile([128, nd * 128], BF16, tag="trp")
            for q in range(nd):
                nc.tensor.transpose(
                    tp[:, q * 128:(q + 1) * 128],
                    xn[:, t * d + q * 128: t * d + (q + 1) * 128], ident[:])
            xt = const.tile([128, nd * 128], BF16, tag=f"xTt{t}",
                            name=f"xTt{t}")
            nc.vector.tensor_copy(xt[:], tp[:])
            xTt.append(xt)

        mins = [const.tile([128, JG * 128], F32, tag=f"mins{t}",
                           name=f"mins{t}") for t in range(nb)]
        otiles = [out_pool.tile([128, n_out + 7], F32, tag=f"o{t}",
                                name=f"o{t}") for t in range(nb)]

        # ---- main loop: pairs of j-groups; W tiles stay resident ----
        n_pairs = JG // 2
        for pr in range(n_pairs):
            j0 = pr * 1024
            ctp = const.tile([128, nd * 1024], BF16, tag=f"ct{pr}",
                             name=f"ct{pr}")
            # W rides the ACT HWDGE ring so the small const loads on the SP
            # ring never delay it
            nc.scalar.dma_start(
                out=ctp[:].rearrange("p (q j) -> p q j", j=1024),
                in_=w_d.rearrange("(q p) j -> p q j", p=128)[:, :, j0:j0 + 1024],
            )
            for t in range(nb):
                pms = [mmp.tile([128, 512], F32, tag="mm", name=f"pm{pr}_{t}_{h}")
                       for h in range(2)]
                for q in range(nd):
                    # one stationary load serves both j-groups of the pair
                    for h in range(2):
                        nc.tensor.matmul(
                            pms[h][:],
                            lhsT=xTt[t][:, q * 128:(q + 1) * 128],
                            rhs=ctp[:, q * 1024 + h * 512: q * 1024 + (h + 1) * 512],
                            start=(q == 0), stop=False)
                for h in range(2):
                    jg = 2 * pr + h
                    nc.tensor.matmul(
                        pms[h][:], lhsT=corr_lhsT[:, t * 128:(t + 1) * 128],
                        rhs=corr_rhs[:, jg * 512:(jg + 1) * 512],
                        start=False, stop=True)
                    nc.vector.tensor_reduce(
                        out=mins[t][:, jg * 128:(jg + 1) * 128],
                        in_=pms[h][:].rearrange("p (c n) -> p c n", n=npc),
                        axis=AX.X, op=ALU.min)
                # stream y = -sqrt(d2min) for this pair's classes now, so the
                # kernel tail only handles the last block + soft column
                c_lo = pr * 256
                c_hi = min((pr + 1) * 256, n_classes)
                if c_hi > c_lo:
                    nc.scalar.activation(otiles[t][:, c_lo:c_hi],
                                         mins[t][:, c_lo:c_hi], AF.Sqrt,
                                         bias=0.0, scale=1.0)
                    nc.vector.tensor_scalar_mul(otiles[t][:, c_lo:c_hi],
                                                otiles[t][:, c_lo:c_hi], -1.0)
                if pr == n_pairs - 1:
                    eng = nc.sync if t % 2 == 0 else nc.scalar
                    eng.dma_start(out_d[t * 128:(t + 1) * 128, 0:n_classes],
                                  otiles[t][:, 0:n_classes])

        # ---- epilogue: just the soft_accept column ----
        ssall = const.tile([128, nb], F32)
        softall = const.tile([128, nb], F32)
        for t in range(nb):
            # min over classes (garbage classes hold ~1e9, never win)
            mmc = small.tile([128, 1], F32, tag="mmc")
            nc.vector.tensor_reduce(out=mmc[:], in_=mins[t][:], axis=AX.X,
                                    op=ALU.min)
            s0 = small.tile([128, 1], F32, tag="s0")
            nc.scalar.activation(s0[:], mmc[:], AF.Sqrt, bias=0.0, scale=1.0)
            # one Newton step: s1 = (s0 + v/s0)/2  (ACT sqrt is low precision)
            rc = small.tile([128, 1], F32, tag="rc")
            nc.vector.reciprocal(rc[:], s0[:])
            t1 = small.tile([128, 1], F32, tag="t1")
            nc.vector.tensor_tensor(out=t1[:], in0=mmc[:], in1=rc[:],
                                    op=ALU.mult)
            nc.vector.tensor_tensor(out=t1[:], in0=s0[:], in1=t1[:],
                                    op=ALU.add)
            nc.vector.tensor_scalar_mul(ssall[:, t:t + 1], t1[:], 0.5)
        # soft_accept = sigmoid(min_dist * (-1/temp) + max_ac/temp)
        nc.scalar.activation(softall[:], ssall[:], AF.Sigmoid,
                             bias=acol[:], scale=bcol[:])
        for t in range(nb):
            nc.gpsimd.dma_start(
                out_d[t * 128:(t + 1) * 128, n_classes:n_classes + 1],
                softall[:, t:t + 1])

    nc.compile()
    return nc


_CACHE = {}


def _get_nc():
    if "nc" not in _CACHE:
        _CACHE["nc"] = build_nc()
    return _CACHE["nc"]


def _prep_centroids(c):
    """Weight pre-packing: W = -2*c^T (zero-padded to 4096 cols) and the
    [c2_hi, c2_lo, 1, 1] bf16 correction rows (1e9 on padded classes)."""
    cnp = math.ceil(CN / 512) * 512
    w = np.zeros((D, cnp), dtype=ml_dtypes.bfloat16)
    w[:, :CN] = (np.ascontiguousarray(c.T) * np.float32(-2.0)).astype(
        ml_dtypes.bfloat16)
    c2 = (c.astype(np.float64) ** 2).sum(1).astype(np.float32)
    c2f = np.full(cnp, GARBAGE_C2, dtype=np.float32)
    c2f[:CN] = c2
    c2h = c2f.astype(ml_dtypes.bfloat16)
    c2l = (c2f - c2h.astype(np.float32)).astype(ml_dtypes.bfloat16)
    ones = np.ones(cnp, dtype=ml_dtypes.bfloat16)
    c2r = np.stack([c2h, c2l, ones, ones], axis=0)
    return w, c2r


def _host_prep(x, centroids, std_scale, ac_temp, running_mean, running_var):
    x = np.asarray(x, dtype=np.float32)
    c = np.asarray(centroids, dtype=np.float32).reshape(CN, D)
    std_scale = np.float32(np.asarray(std_scale))
    ac_temp = np.float32(np.asarray(ac_temp))
    running_mean = np.float32(np.asarray(running_mean))
    running_var = np.float32(np.asarray(running_var))

    clip = np.float32(min(max(float(std_scale), 0.0), AC_STD_LIM))
    max_ac = np.float32(running_mean + clip * np.float32(np.sqrt(running_var)))
    acol = np.full((128, 1), np.float32(max_ac / ac_temp), dtype=np.float32)
    bcol = np.full((128, 1), np.float32(-1.0 / ac_temp), dtype=np.float32)

    w, c2r = _prep_centroids(c)

    b_loc = B // N_CORES
    in_maps = []
    for i in range(N_CORES):
        in_maps.append({
            "x": np.ascontiguousarray(x[i * b_loc:(i + 1) * b_loc]),
            "wt": w,
            "c2r": c2r,
            "acol": acol,
            "bcol": bcol,
        })
    return in_maps


def run_spmd(in_maps, trace=False, **kw):
    nc = _get_nc()
    return run_bass_kernel_spmd(nc, in_maps, list(range(N_CORES)),
                                trace=trace, **kw)


def kernel(x, centroids, std_scale, ac_temp, running_mean, running_var):
    in_maps = _host_prep(x, centroids, std_scale, ac_temp,
                         running_mean, running_var)
    res = run_spmd(in_maps)
    return np.concatenate([res.results[i]["out"] for i in range(N_CORES)],
                          axis=0)


# revision 36
# speedup vs baseline: 1.1271x; 1.0080x over previous
"""CentroidLayer (retrieval kNN) Bass/Tile kernel for 8 trn2 NeuronCores.

Sharding: data-parallel over batch B (4096 -> 512 rows/core); centroids
replicated (they are module weights, so their layout prep — transpose to
contraction-major and the per-centroid squared-norm rows — happens on the
host once, like any weight pre-packing).

Per core (measured ~101-105 us on HW; bf16 matmul floor is ~53 us/core,
fixed kernel preamble+teardown is ~14 us of the gap):
  - W = -2*c^T (host-prepped bf16) loads fully resident into SBUF (8 MB)
    on the ACT HWDGE ring; x is gpsimd cast-loaded per b-tile and
    PE-transposed to D-major (contraction dim on partitions)
  - d2 = x2 + c2 - 2*x@c^T accumulates in PSUM over 8 K=128 bf16 matmuls
    plus one K=4 correction matmul with rows [1,1,x2_hi,x2_lo] x
    [c2_hi,c2_lo,1,1] (hi/lo bf16 splits keep f32-level accuracy); x2 is
    computed on device via ACT Square+accumulate
  - the matmul loop shares each stationary operand across both j-groups of
    a resident W pair (fewer LDWEIGHTS, denser PE stream)
  - grouped min over the 4 centroids per class on DVE (tensor_reduce min)
  - y = -sqrt(d2min) (ACT sqrt + DVE negate) streams into the output tiles
    per pair during the matmul body; the kernel tail only computes the
    soft_accept column: sigmoid(min_dist*(-1/temp) + max_ac/temp) with a
    Newton-refined sqrt for the [128,1] min-distance column (ACT sqrt
    tables are low precision; the sigmoid is sensitive to absolute error)
Outputs [512, 1001] f32 per core are concatenated on host.
"""

import math
from contextlib import ExitStack

import numpy as np
import ml_dtypes

import concourse.bacc as bacc
import concourse.bass as bass
import concourse.mybir as mybir
import concourse.tile as tile
from concourse.bass_utils import run_bass_kernel_spmd
from concourse.masks import make_identity

F32 = mybir.dt.float32
BF16 = mybir.dt.bfloat16
AF = mybir.ActivationFunctionType
ALU = mybir.AluOpType
AX = mybir.AxisListType

N_CORES = 8
B, D = 4096, 1024
C_CLASSES, NPC = 1000, 4
CN = C_CLASSES * NPC
AC_STD_LIM = 5.0
GARBAGE_C2 = 1.0e9


def build_nc(b_loc=B // N_CORES, cn=CN, d=D, npc=NPC, n_classes=C_CLASSES,
             n_cores=N_CORES):
    """Build + compile the per-core Bass module (SPMD: same program on all
    cores; only the x shard differs)."""
    assert b_loc % 128 == 0 and d % 128 == 0 and cn % npc == 0
    nb, nd = b_loc // 128, d // 128
    JG = math.ceil(cn / 512)          # j-groups of 512 centroid rows
    assert JG % 2 == 0
    cnp = JG * 512                    # padded centroid rows
    n_out = n_classes + 1

    nc = bacc.Bacc("TRN2", target_bir_lowering=False, debug=False,
                   enable_asserts=False, num_devices=n_cores)

    x_d = nc.dram_tensor("x", [b_loc, d], F32, kind="ExternalInput").ap()
    w_d = nc.dram_tensor("wt", [d, cnp], BF16, kind="ExternalInput").ap()
    c2_d = nc.dram_tensor("c2r", [4, cnp], BF16, kind="ExternalInput").ap()
    a_d = nc.dram_tensor("acol", [128, 1], F32, kind="ExternalInput").ap()
    bi_d = nc.dram_tensor("bcol", [128, 1], F32, kind="ExternalInput").ap()
    out_d = nc.dram_tensor("out", [b_loc, n_out], F32, kind="ExternalOutput").ap()

    with tile.TileContext(nc) as tc, ExitStack() as ctx:
        const = ctx.enter_context(tc.tile_pool(name="const", bufs=1))
        sq_pool = ctx.enter_context(tc.tile_pool(name="sq", bufs=2))
        small = ctx.enter_context(tc.tile_pool(name="small", bufs=2))
        out_pool = ctx.enter_context(tc.tile_pool(name="otile", bufs=1))
        trp = ctx.enter_context(tc.tile_pool(name="trp", bufs=2, space="PSUM"))
        mmp = ctx.enter_context(tc.tile_pool(name="mmp", bufs=5, space="PSUM"))
        smp = ctx.enter_context(tc.tile_pool(name="smp", bufs=1, space="PSUM"))

        ident = const.tile([128, 128], BF16)
        make_identity(nc, ident)
        # HAM warm-up: the PE clock-gate only opens after ~3.4us of real
        # matmul activity (transposes don't count), so burn dummy identity
        # matmuls while the x/W DMAs are in flight; otherwise the first
        # ~3.4us of the main stream runs at half clock
        warm = trp.tile([128, 512], F32, tag="trp", name="warm")
        for i in range(26):
            nc.tensor.matmul(warm[:, 0:128], lhsT=ident[:], rhs=ident[:],
                             start=True, stop=True)
        acol = const.tile([128, 1], F32)
        nc.sync.dma_start(acol[:], a_d)
        bcol = const.tile([128, 1], F32)
        nc.sync.dma_start(bcol[:], bi_d)
        corr_rhs = const.tile([4, cnp], BF16)
        nc.sync.dma_start(corr_rhs[:], c2_d)

        # ---- x prep: per-b-tile cast-load, x2 (+hi/lo), transpose ----
        xn = const.tile([128, nb * d], BF16)       # bt-major natural x, bf16
        x2c = const.tile([128, nb], F32)           # x2 per b-tile column
        corr_lhsT = const.tile([4, nb * 128], BF16)
        xTt = []                                   # per-b-tile [d-part, q*128]
        for t in range(nb):
            nc.gpsimd.dma_start(out=xn[:, t * d:(t + 1) * d],
                                in_=x_d[t * 128:(t + 1) * 128, :])
            xsq = sq_pool.tile([128, d], F32, tag="xsq")
            nc.scalar.activation(xsq[:], xn[:, t * d:(t + 1) * d], AF.Square,
                                 accum_out=x2c[:, t:t + 1])
            # cols [1, 1, x2_hi, x2_lo], then one 128x4 -> 4x128 transpose
            hl = small.tile([128, 4], BF16, tag="hl")
            nc.vector.memset(hl[:, 0:2], 1.0)
            nc.vector.tensor_copy(hl[:, 2:3], x2c[:, t:t + 1])
            hf = small.tile([128, 1], F32, tag="hf")
            nc.vector.tensor_copy(hf[:], hl[:, 2:3])
            nc.vector.tensor_tensor(out=hf[:], in0=x2c[:, t:t + 1],
                                    in1=hf[:], op=ALU.subtract)
            nc.vector.tensor_copy(hl[:, 3:4], hf[:])
            sp2 = smp.tile([8, 128], BF16, tag="sp")
            nc.tensor.transpose(sp2[0:4, :], hl[:], ident[:])
            nc.vector.tensor_copy(corr_lhsT[:, t * 128:(t + 1) * 128],
                                  sp2[0:4, :])
            # transpose this b-tile to D-major: 8 blocks into one psum bank
            tp = trp.tile([128, nd * 128], BF16, tag="trp")
            for q in range(nd):
                nc.tensor.transpose(
                    tp[:, q * 128:(q + 1) * 128],
                    xn[:, t * d + q * 128: t * d + (q + 1) * 128], ident[:])
            xt = const.tile([128, nd * 128], BF16, tag=f"xTt{t}",
                            name=f"xTt{t}")
            nc.vector.tensor_copy(xt[:], tp[:])
            xTt.append(xt)
            # keep the HAM activity window alive through the transpose phase
            wt2 = trp.tile([128, 512], F32, tag="trp", name=f"warm{t}")
            for i in range(6):
                nc.tensor.matmul(wt2[:, 0:128], lhsT=ident[:], rhs=ident[:],
                                 start=True, stop=True)

        mins = [const.tile([128, JG * 128], F32, tag=f"mins{t}",
                           name=f"mins{t}") for t in range(nb)]
        otiles = [out_pool.tile([128, n_out + 7], F32, tag=f"o{t}",
                                name=f"o{t}") for t in range(nb)]

        # ---- main loop: pairs of j-groups; W tiles stay resident ----
        n_pairs = JG // 2
        for pr in range(n_pairs):
            j0 = pr * 1024
            ctp = const.tile([128, nd * 1024], BF16, tag=f"ct{pr}",
                             name=f"ct{pr}")
            # W rides the ACT HWDGE ring so the small const loads on the SP
            # ring never delay it
            nc.scalar.dma_start(
                out=ctp[:].rearrange("p (q j) -> p q j", j=1024),
                in_=w_d.rearrange("(q p) j -> p q j", p=128)[:, :, j0:j0 + 1024],
            )
            for t in range(nb):
                pms = [mmp.tile([128, 512], F32, tag="mm", name=f"pm{pr}_{t}_{h}")
                       for h in range(2)]
                for q in range(nd):
                    # one stationary load serves both j-groups of the pair
                    for h in range(2):
                        nc.tensor.matmul(
                            pms[h][:],
                            lhsT=xTt[t][:, q * 128:(q + 1) * 128],
                            rhs=ctp[:, q * 1024 + h * 512: q * 1024 + (h + 1) * 512],
                            start=(q == 0), stop=False)
                for h in range(2):
                    jg = 2 * pr + h
                    nc.tensor.matmul(
                        pms[h][:], lhsT=corr_lhsT[:, t * 128:(t + 1) * 128],
                        rhs=corr_rhs[:, jg * 512:(jg + 1) * 512],
                        start=False, stop=True)
                    nc.vector.tensor_reduce(
                        out=mins[t][:, jg * 128:(jg + 1) * 128],
                        in_=pms[h][:].rearrange("p (c n) -> p c n", n=npc),
                        axis=AX.X, op=ALU.min)
                # stream y = -sqrt(d2min) for this pair's classes now, so the
                # kernel tail only handles the last block + soft column
                c_lo = pr * 256
                c_hi = min((pr + 1) * 256, n_classes)
                if c_hi > c_lo:
                    nc.scalar.activation(otiles[t][:, c_lo:c_hi],
                                         mins[t][:, c_lo:c_hi], AF.Sqrt,
                                         bias=0.0, scale=1.0)
                    nc.vector.tensor_scalar_mul(otiles[t][:, c_lo:c_hi],
                                                otiles[t][:, c_lo:c_hi], -1.0)
                if pr == n_pairs - 1:
                    eng = nc.sync if t % 2 == 0 else nc.scalar
                    eng.dma_start(out_d[t * 128:(t + 1) * 128, 0:n_classes],
                                  otiles[t][:, 0:n_classes])

        # ---- epilogue: just the soft_accept column ----
        ssall = const.tile([128, nb], F32)
        softall = const.tile([128, nb], F32)
        for t in range(nb):
            # min over classes (garbage classes hold ~1e9, never win)
            mmc = small.tile([128, 1], F32, tag="mmc")
            nc.vector.tensor_reduce(out=mmc[:], in_=mins[t][:], axis=AX.X,
                                    op=ALU.min)
            s0 = small.tile([128, 1], F32, tag="s0")
            nc.scalar.activation(s0[:], mmc[:], AF.Sqrt, bias=0.0, scale=1.0)
            # one Newton step: s1 = (s0 + v/s0)/2  (ACT sqrt is low precision)
            rc = small.tile([128, 1], F32, tag="rc")
            nc.vector.reciprocal(rc[:], s0[:])
            t1 = small.tile([128, 1], F32, tag="t1")
            nc.vector.tensor_tensor(out=t1[:], in0=mmc[:], in1=rc[:],
                                    op=ALU.mult)
            nc.vector.tensor_tensor(out=t1[:], in0=s0[:], in1=t1[:],
                                    op=ALU.add)
            nc.vector.tensor_scalar_mul(ssall[:, t:t + 1], t1[:], 0.5)
        # soft_accept = sigmoid(min_dist * (-1/temp) + max_ac/temp)
        nc.scalar.activation(softall[:], ssall[:], AF.Sigmoid,
                             bias=acol[:], scale=bcol[:])
        for t in range(nb):
            nc.gpsimd.dma_start(
                out_d[t * 128:(t + 1) * 128, n_classes:n_classes + 1],
                softall[:, t:t + 1])

    nc.compile()
    return nc


_CACHE = {}


def _get_nc():
    if "nc" not in _CACHE:
        _CACHE["nc"] = build_nc()
    return _CACHE["nc"]


def _prep_centroids(c):
    """Weight pre-packing: W = -2*c^T (zero-padded to 4096 cols) and the
    [c2_hi, c2_lo, 1, 1] bf16 correction rows (1e9 on padded classes)."""
    cnp = math.ceil(CN / 512) * 512
    w = np.zeros((D, cnp), dtype=ml_dtypes.bfloat16)
    w[:, :CN] = (np.ascontiguousarray(c.T) * np.float32(-2.0)).astype(
        ml_dtypes.bfloat16)
    c2 = (c.astype(np.float64) ** 2).sum(1).astype(np.float32)
    c2f = np.full(cnp, GARBAGE_C2, dtype=np.float32)
    c2f[:CN] = c2
    c2h = c2f.astype(ml_dtypes.bfloat16)
    c2l = (c2f - c2h.astype(np.float32)).astype(ml_dtypes.bfloat16)
    ones = np.ones(cnp, dtype=ml_dtypes.bfloat16)
    c2r = np.stack([c2h, c2l, ones, ones], axis=0)
    return w, c2r


def _host_prep(x, centroids, std_scale, ac_temp, running_mean, running_var):
    x = np.asarray(x, dtype=np.float32)
    c = np.asarray(centroids, dtype=np.float32).reshape(CN, D)
    std_scale = np.float32(np.asarray(std_scale))
    ac_temp = np.float32(np.asarray(ac_temp))
    running_mean = np.float32(np.asarray(running_mean))
    running_var = np.float32(np.asarray(running_var))

    clip = np.float32(min(max(float(std_scale), 0.0), AC_STD_LIM))
    max_ac = np.float32(running_mean + clip * np.float32(np.sqrt(running_var)))
    acol = np.full((128, 1), np.float32(max_ac / ac_temp), dtype=np.float32)
    bcol = np.full((128, 1), np.float32(-1.0 / ac_temp), dtype=np.float32)

    w, c2r = _prep_centroids(c)

    b_loc = B // N_CORES
    in_maps = []
    for i in range(N_CORES):
        in_maps.append({
            "x": np.ascontiguousarray(x[i * b_loc:(i + 1) * b_loc]),
            "wt": w,
            "c2r": c2r,
            "acol": acol,
            "bcol": bcol,
        })
    return in_maps


def run_spmd(in_maps, trace=False, **kw):
    nc = _get_nc()
    return run_bass_kernel_spmd(nc, in_maps, list(range(N_CORES)),
                                trace=trace, **kw)


def kernel(x, centroids, std_scale, ac_temp, running_mean, running_var):
    in_maps = _host_prep(x, centroids, std_scale, ac_temp,
                         running_mean, running_var)
    res = run_spmd(in_maps)
    return np.concatenate([res.results[i]["out"] for i in range(N_CORES)],
                          axis=0)
